# revision 40
# baseline (speedup 1.0000x reference)
"""Single-head causal attention on 8 TRN2 NeuronCores (Bass/Tile).

Problem: x [4, 2048, 1024] fp32; wq/wk/wv [1024, 128]; wo [128, 1024].
out = softmax_causal((x@wq)(x@wk)^T / sqrt(128)) @ (x@wv) @ wo

Sharding: 8 cores = 4 batches x 2 query-interleavings (zebra pattern:
core parity p takes seq blocks {4g+o0, 4g+o1} of each group g of 4
blocks). Host permutes+transposes x so each core's query slots are
contiguous; slot j = permuted q cols [512j : 512j+256] = seq blocks
{4j+order[0], 4j+order[1]}, attending permuted key prefix [0 : 512(j+1)].

Schedule (v2, restructured from the 67us baseline):
 - Phase order: warmup -> pjA -> trA -> attA -> (outsA -> pjB) -> trB
   -> attB(+early lo outs) -> outsB.  attention pair 0 only needs
   seq-half-0 K/V/Q, so it runs right after pjA; x half 1's DMA hides
   entirely under attA. Pair-A out blocks run before pjB (their scale
   copies must clear DVE/ACT early: attB needs all 8 psum banks and
   putting them after pjB measured +8us via bank-WAR on the last
   copy). Pair-B's slot-2 den/ctx are final one key-group early, so
   out blocks 4-5 run during group 3; only blocks 6-7 trail.
 - x DMA: per-chunk [128, 1024] tiles. Measured: the two HWDGE queues
   SHARE a ~358 GB/s per-core HBM cap (184 GB/s each when both
   stream), so weights+half0 (2.75MB) cannot land before ~+8us of DMA
   start no matter the split. wv leads scalar, wq+wk lead sync, then
   half-0 chunks 0-3 (scalar) / 4-7 (sync); consumption order
   [0,1,4,2,5,3,6,7] tracks arrival. mask+wo follow on scalar, then
   x half 1 on both queues (needed only by pjB at ~+18us).
 - HAM/DVFS laws (all measured on this part):
     * PE clock ramps 0.8 -> ~1.7 -> 2.4 GHz only under a DENSE
       uninterrupted matmul stream; ~1us+ PE dips in the first ~10us
       after lift re-throttle to half clock for 3.4us+.
     * WARM_N=12 dummy matmuls bridge DMA startup (~2.4us latency +
       transfers); too few (8) left a gap -> +12us.
     * Dep-free "bridge" dummies pad every inter-phase seam (copy
       latencies) to hold the clock; they are free when the seam is
       real idle. NEVER interleave them INSIDE a projection
       accumulation stream: that pinned the whole run at the ~1.7GHz
       p-state (+15us).
     * The HAM k-field gates only the PE clock; ACT/DVE copy
       durations are unaffected by the k=4 tail drop.
     * After ~1.5h of continuous benching the part thermally derates
       ~20% on PE (218 -> 261ns per 512-col matmul); compare configs
       interleaved or after idle.
 - fp8 was evaluated and is numerically DEAD here (budget 2e-2):
   e4m3 x/w for projections -> 8.3e-2 (per-query rank-1 score error,
   softmax does not wash it); p/v in fp8 with rebiased exp -> 2.5e-2+.
   DoubleRow also cannot apply to scores/AV (contraction dim is 128).
 - Projections per seq-half accumulate d-chunk-outer in 5 psum banks;
   psum->SBUF copies split DVE/ACT, vt first (trA gates on it), qt
   before kt1 (next pair's first score group gates on qt+kt0).
 - V transposes: 4 PE transposes into one [128,512] f16 psum tile, ONE
   copy out (4 copies total per half pair vs 16 [128,128] copies).
 - Attention processes slot PAIRS ({0,1}, {2,3}): shared key groups
   stream 512-wide q, score tiles [P,1024] (2 banks, bufs=2), ONE exp
   per tile. Causal masks are ADDITIVE (-30000) applied by PE matmul
   through an identity lhsT. den/AV skip all-masked regions.
 - den partition-ified by [1,128]->[128,1] PE transpose-matmuls into
   psum, then DVE reciprocals (pair B split lo/hi into separate rdenB
   tiles to avoid false WARs).
 - Output projection: 1/den fused into the psum->SBUF copies, split
   ACT/DVE; stores on the sync queue (idle after x half 1), tail
   blocks split across both queues; the last block stores per-half.
   Pair-B out blocks get their own ot tiles (bufs=4) so no copy waits
   on a prior store's DMA.
"""

import numpy as np

import concourse.bass as bass
from concourse import bacc
import concourse.mybir as mybir
import concourse.tile as tile
from concourse.bass_utils import run_bass_kernel_spmd
from concourse.masks import make_identity

F32 = mybir.dt.float32
F16 = mybir.dt.float16

D_MODEL = 1024
D_HEAD = 128
SEQ = 2048
BATCH = 4
NCORES = 8
P = 128            # partitions / block size
DC = D_MODEL // P  # 8 d_model chunks
NB = SEQ // P      # 16 seq blocks
NSLOT = 4
QW = 256           # queries per slot
NQ = NSLOT * QW    # 1024 queries per core
HS = SEQ // 2      # seq half
NG = DC // 2       # 4 chunk-pair granules per half
SCALE = 1.0 / float(np.sqrt(D_HEAD))
EXP_BIAS = -3.0    # exp(scale*s - 3): keeps exp outputs < 1e4 (fp16-safe)
MASK_NEG = -30000.0
WARM_N = 12        # dummy PE matmuls at t=0 (HAM clock ramp + DMA-wait).
                   # 8 was tried: the ramp did not complete before the
                   # DMA-paced phase's PE gaps and full clock arrived
                   # 5us late (+12us total). Overshoot is cheap (213ns
                   # per dummy warm), undershoot costs ~3.4us; keep >=10.


def block_order(parity: int) -> list[int]:
    order = []
    for g in range(4):
        if parity == 0:
            order += [4 * g, 4 * g + 3, 4 * g + 1, 4 * g + 2]
        else:
            order += [4 * g + 1, 4 * g + 2, 4 * g, 4 * g + 3]
    return order


def make_mask01(parity: int) -> np.ndarray:
    """0/1 keep-mask for the diagonal 512-key group of a slot,
    transposed: [512 k, 256 q]."""
    P4 = block_order(parity)[:4]
    m = np.zeros((512, 256), dtype=np.float16)
    kr = np.arange(P)[:, None]
    qc = np.arange(P)[None, :]
    tri = (kr <= qc).astype(np.float16)
    for kb2 in range(4):
        K = P4[kb2]
        for qb2 in range(2):
            Q = P4[qb2]
            blk = m[P * kb2:P * (kb2 + 1), P * qb2:P * (qb2 + 1)]
            if K < Q:
                blk[:] = 1.0
            elif K > Q:
                blk[:] = 0.0
            else:
                blk[:] = tri
    return m


def _attention_kernel(tc: tile.TileContext, xt_d, wq_d, wk_d, wv_d, wo_d,
                      maskt_d, out_d):
    nc = tc.nc

    with (
        tc.tile_pool(name="const", bufs=1) as const_pool,
        tc.tile_pool(name="big", bufs=1) as big_pool,
        tc.tile_pool(name="ptp", bufs=6) as pt_pool,
        tc.tile_pool(name="outA", bufs=2) as outA_pool,
        tc.tile_pool(name="outB", bufs=4) as outB_pool,
    ):
        # ---- DMA plan (each HWDGE queue ~184 GB/s; x chunk = 256KB =
        # ~1.4us):
        #   scalar: wv -> x0 c0-c3 -> mask -> wo -> x1 c0-c3
        #   sync:   wq,wk -> x0 c4-c7 -> x1 c4-c7 -> out stores
        # All three weights land by ~+2.8us of queue start (they gate
        # chunk 0's q/k/v matmuls; a single-queue wq,wk,wv chain made
        # chunk-0's vt wait until ~13us and the resulting PE gap
        # re-throttled the HAM clock). pjA consumes chunks alternating
        # between the queues (0,4,1,5,...) so it runs PE-paced. ----
        wq_sb = const_pool.tile([P, DC, P], F16)
        nc.sync.dma_start(out=wq_sb, in_=wq_d.rearrange("p (c h) -> p c h", h=P))
        wk_sb = const_pool.tile([P, DC, P], F16)
        nc.sync.dma_start(out=wk_sb, in_=wk_d.rearrange("p (c h) -> p c h", h=P))
        wv_sb = const_pool.tile([P, DC, P], F16)
        nc.scalar.dma_start(out=wv_sb, in_=wv_d.rearrange("p (c h) -> p c h", h=P))

        xc = [[None] * 2 for _ in range(DC)]   # [chunk][half]
        for c in range(4):
            t = big_pool.tile([P, HS], F16, name=f"xc{c}_0")
            nc.scalar.dma_start(out=t, in_=xt_d[P * c:P * (c + 1), 0:HS])
            xc[c][0] = t
        for c in range(4, DC):
            t = big_pool.tile([P, HS], F16, name=f"xc{c}_0")
            nc.sync.dma_start(out=t, in_=xt_d[P * c:P * (c + 1), 0:HS])
            xc[c][0] = t

        maskt_sb = const_pool.tile([P, 4, QW], F16)
        nc.scalar.dma_start(out=maskt_sb,
                            in_=maskt_d.rearrange("p (b q) -> p b q", q=QW))
        wo_sb = const_pool.tile([P, D_MODEL], F16)
        nc.scalar.dma_start(out=wo_sb, in_=wo_d)

        for c in range(DC):
            t = big_pool.tile([P, HS], F16, name=f"xc{c}_1")
            eng = nc.scalar if c < 4 else nc.sync
            eng.dma_start(out=t, in_=xt_d[P * c:P * (c + 1), HS:SEQ])
            xc[c][1] = t

        def xchunk(c, h):
            return xc[c][h]

        # ---- constants ----
        # warm_sb on gpsimd: its memsets run at ~6.0us (right after the
        # preamble barrier) vs DVE's ~7.4us, and the warmup must not
        # wait for make_identity's gpsimd iota chain either -- so the
        # warmup matmuls use warm_sb as BOTH lhsT and rhs.
        warm_sb = const_pool.tile([P, 512], F16)
        nc.gpsimd.memset(warm_sb, 0.001)
        ident = const_pool.tile([P, P], F16)
        make_identity(nc, ident)
        ones = const_pool.tile([P, 1], F16)
        nc.vector.memset(ones, 1.0)
        expbias = const_pool.tile([P, 1], F32)
        nc.vector.memset(expbias, EXP_BIAS)
        # dummy exp pulls the ACT exp table load early -- but it is
        # emitted AFTER the DMA section so the ~1.3us table load does
        # not delay the scalar queue's x-chunk enqueues.
        actwarm = const_pool.tile([P, 1], F32)
        nc.scalar.activation(out=actwarm, in_=expbias,
                             func=mybir.ActivationFunctionType.Exp)

        # per-quarter / per-pair SBUF tensors (fine split: Tile tracks
        # deps at whole-tile granularity)
        qt_h = [big_pool.tile([P, 512], F16, name=f"qt{h}") for h in range(2)]
        kt_q = [big_pool.tile([P, 512], F16, name=f"kt{q}") for q in range(4)]
        vt_q = [big_pool.tile([P, 512], F16, name=f"vt{q}") for q in range(4)]
        v_h = [big_pool.tile([P, HS], F16, name=f"v{h}") for h in range(2)]
        ctxt_p = [big_pool.tile([P, 512], F16, name=f"ctxt{a}") for a in range(2)]
        den_p = [big_pool.tile([1, 512], F16, name=f"den{a}") for a in range(2)]
        rden_p = [big_pool.tile([P, 4], F32, name=f"rden{a}") for a in range(2)]
        # pair B's reciprocals in separate lo/hi tiles: recip-hi must
        # not carry a false WAR on the lo out-blocks' scalar reads
        rdenB = [big_pool.tile([P, 2], F32, name=f"rdenB{i}") for i in range(2)]

        def kt_blk(kb):
            return kt_q[kb // 4][:, P * (kb % 4):P * (kb % 4 + 1)]

        def v_blk(kb):
            return v_h[kb // 8][:, P * (kb % 8):P * (kb % 8 + 1)]

        def proj_half(h, pj, gap_cb=None):
            """QT/KT/VT for seq half h, d-chunk outer; copies to SBUF.
            gap_cb(ci) may emit PE filler between chunks (the early
            chunks of half 0 arrive slower than PE consumes them; an
            unpadded >=1us PE dip right after the HAM clock lift
            re-throttles the core)."""
            qt_ps = pj.tile([P, 512], F32, name=f"qt_ps{h}")
            kt_ps = [pj.tile([P, 512], F32, name=f"kt_ps{h}_{i}")
                     for i in range(2)]
            vt_ps = [pj.tile([P, 512], F32, name=f"vt_ps{h}_{i}")
                     for i in range(2)]
            # chunk order matches measured arrival: the two queues share
            # the ~358 GB/s per-core HBM cap (~1.43us per 256KB chunk
            # per queue when both stream); scalar leads with wv (1
            # chunk) vs sync's wq+wk (2), so scalar's chunks run ~1
            # transfer ahead
            corder = [0, 1, 4, 2, 5, 3, 6, 7]
            for ci, c in enumerate(corder):
                if gap_cb is not None:
                    gap_cb(ci)
                xh = xchunk(c, h)
                xr = xh.rearrange("p (g q) -> p g q", q=QW)
                st, sp = (ci == 0), (ci == DC - 1)
                nc.tensor.matmul(qt_ps, lhsT=wq_sb[:, c, :],
                                 rhs=xr[:, 0:3:2, :], start=st, stop=sp,
                                 skip_group_check=True)
                for i in range(2):
                    nc.tensor.matmul(kt_ps[i], lhsT=wk_sb[:, c, :],
                                     rhs=xh[:, 512 * i:512 * (i + 1)],
                                     start=st, stop=sp, skip_group_check=True)
                for i in range(2):
                    nc.tensor.matmul(vt_ps[i], lhsT=wv_sb[:, c, :],
                                     rhs=xh[:, 512 * i:512 * (i + 1)],
                                     start=st, stop=sp, skip_group_check=True)
            # copies split DVE/ACT. Half 0: vt first (trA gates on it),
            # then qt+kt0 (attA's first score group). Half 1: qt FIRST
            # -- pair 1's first TWO score groups read half-0 keys
            # (kt_q[0/1], long resident) and gate only on qt_h[1];
            # v_h[1]/kt_q[2,3] aren't touched until denav(2)/st(2).
            if h == 0:
                nc.vector.tensor_copy(vt_q[0], vt_ps[0])
                nc.scalar.copy(vt_q[1], vt_ps[1])
                nc.vector.tensor_copy(qt_h[0], qt_ps)
                nc.scalar.copy(kt_q[0], kt_ps[0])
                nc.vector.tensor_copy(kt_q[1], kt_ps[1])
            else:
                nc.vector.tensor_copy(qt_h[1], qt_ps)
                nc.scalar.copy(vt_q[3], vt_ps[1])
                nc.vector.tensor_copy(vt_q[2], vt_ps[0])
                nc.scalar.copy(kt_q[2], kt_ps[0])
                nc.vector.tensor_copy(kt_q[3], kt_ps[1])

        def transposes(h, trp):
            # 4 PE transposes into one [128,512] f16 psum tile, then a
            # single copy: 4 copies total per half vs 16 [128,128] ones.
            # Half 1 starts with quarter 1: its vt copy (ACT, first in
            # queue) lands ~0.7us before vt2 (2nd on DVE behind qt).
            for q4 in ((0, 1) if h == 0 else (1, 0)):
                ptr = trp.tile([P, 512], F16, tag="tr", bufs=2,
                               name=f"ptr{h}_{q4}")
                for b4 in range(4):
                    nc.tensor.transpose(
                        ptr[:, P * b4:P * (b4 + 1)],
                        vt_q[2 * h + q4][:, P * b4:P * (b4 + 1)], ident)
                if q4 == 0:
                    nc.vector.tensor_copy(v_h[h][:, 0:512], ptr)
                else:
                    nc.scalar.copy(v_h[h][:, 512:1024], ptr)

        def pair_st_exp(a, g, stpool, pt):
            """Scores (+additive mask on diag regions) + exp for pair a,
            key group g -> its own PT tile (recorded in pt)."""
            jlo, jhi = 2 * a, 2 * a + 1
            nfull = jhi
            qt_pair = qt_h[a]
            qt_hi = qt_h[a][:, 256:512]
            wide = g < nfull or g == jlo
            qw = 512 if wide else 256
            qtr = qt_pair if wide else qt_hi
            ptile = pt_pool.tile([P, 2048], F16, tag="pt",
                                 name=f"pt{a}_{g}")
            pt[g] = ptile
            diag = (g == jlo) if wide else True
            nhalf = 2 if wide else 1
            for half in range(nhalf):
                stp = stpool.tile([P, 1024], F32, tag="st", bufs=2,
                                  name=f"st{a}_{g}_{half}")
                for k2h in range(4 // nhalf):
                    k2 = half * 2 + k2h if wide else k2h
                    kb = 4 * g + k2
                    base = qw * k2h  # kb's col base in stp
                    nc.tensor.matmul(stp[:, base:base + qw],
                                     lhsT=kt_blk(kb), rhs=qtr,
                                     start=True, stop=not diag,
                                     skip_group_check=True)
                    if not diag:
                        continue
                    # additive causal mask (identity-matmul accum) on
                    # the diag slot's 256 q. kb1/kb3: q0 sub-block is
                    # all -inf for both parities -> den/av skip it
                    if k2 in (0, 2):
                        nc.tensor.matmul(
                            stp[:, base:base + 256],
                            lhsT=ident, rhs=maskt_sb[:, k2, :],
                            start=False, stop=True,
                            skip_group_check=True)
                    else:
                        nc.tensor.matmul(
                            stp[:, base + P:base + 256],
                            lhsT=ident, rhs=maskt_sb[:, k2, P:QW],
                            start=False, stop=True,
                            skip_group_check=True)
                off = 1024 * half
                nc.scalar.activation(
                    out=ptile[:, off:off + 1024], in_=stp,
                    func=mybir.ActivationFunctionType.Exp,
                    bias=expbias, scale=SCALE)

        def attention_pair(a, att, early_cb=None):
            """Pair a: slots {2a, 2a+1}; pair q = qt_h[a]. Key groups
            0..2a-1 full, group 2a diag-masked on slot-lo, group 2a+1
            slot-hi only. early_cb (if given) is emitted after the
            second-to-last denav: the slot-lo halves of den/ctx are
            final there (the last group only touches slot-hi columns),
            so the lo out-blocks can run during the last group."""
            jlo, jhi = 2 * a, 2 * a + 1
            nfull = jhi            # groups 0..jhi-1 stream 512q

            den_ps = att.tile([1, 512], F32, tag="den", bufs=1,
                              name=f"den_ps{a}")
            ctx_ps = att.tile([P, 512], F32, tag="ctx", bufs=1,
                              name=f"ctx_ps{a}")
            pt = {}

            def st_exp(g):
                pair_st_exp(a, g, att, pt)

            ngroups = jhi + 1
            ndenav = 0

            def denav(g):
                """den+AV for key group g. Skips regions that the causal
                mask provably zeroes for both parities."""
                nonlocal ndenav
                wide = g < nfull or g == jlo
                ptile = pt[g]
                first = (g == 0)
                last = (ndenav == ngroups - 1)
                parts = []  # (kb, pt_off, den_off, width)
                for k2 in range(4):
                    kb = 4 * g + k2
                    if wide:
                        base = 512 * k2
                        if g == jlo and k2 in (1, 3):
                            parts.append((kb, base + P, P, 512 - P))
                        else:
                            parts.append((kb, base, 0, 512))
                    else:
                        base = 256 * k2
                        if k2 in (1, 3):
                            parts.append((kb, base + P, 256 + P, P))
                        else:
                            parts.append((kb, base, 256, 256))
                for i, (kb, po, do, w) in enumerate(parts):
                    nc.tensor.matmul(den_ps[:, do:do + w], lhsT=ones,
                                     rhs=ptile[:, po:po + w],
                                     start=(first and i == 0),
                                     stop=(last and i == len(parts) - 1),
                                     skip_group_check=True)
                for i, (kb, po, do, w) in enumerate(parts):
                    nc.tensor.matmul(ctx_ps[:, do:do + w], lhsT=v_blk(kb),
                                     rhs=ptile[:, po:po + w],
                                     start=(first and i == 0),
                                     stop=(last and i == len(parts) - 1),
                                     skip_group_check=True)
                ndenav += 1

            # interleave: scores run one group ahead of den/av
            st_exp(0)
            for g in range(1, ngroups):
                st_exp(g)
                denav(g - 1)
                if g == ngroups - 1 and early_cb is not None:
                    early_cb(den_ps, ctx_ps)
            denav(ngroups - 1)

            if early_cb is not None:
                # lo half already finished by early_cb; finish hi half.
                # ctxt-hi on ACT (idle after its last exp), den-hi on
                # DVE -- in parallel, so recip-hi lands early.
                nc.scalar.copy(ctxt_p[a][:, 256:512], ctx_ps[:, 256:512])
                nc.vector.tensor_copy(den_p[a][:, 256:512],
                                      den_ps[:, 256:512])
                dent_hi = att.tile([P, 512], F32, tag="op", bufs=2,
                                   name="dent_hi")
                for j in range(2):
                    qb2 = 2 + j
                    nc.tensor.matmul(dent_hi[:, j:j + 1],
                                     lhsT=den_p[a][0:1,
                                                   P * qb2:P * (qb2 + 1)],
                                     rhs=ones[0:1, 0:1],
                                     start=(j == 0), stop=(j == 1),
                                     skip_group_check=True)
                nc.vector.reciprocal(rdenB[1], dent_hi[:, 0:2])
                return

            # finish: ctx/den to SBUF; den partition-ified by PE
            # transpose-matmuls into a psum tile sharing den's bank;
            # then one DVE reciprocal.
            if a == 0:
                nc.vector.tensor_copy(ctxt_p[a], ctx_ps)
            else:
                nc.scalar.copy(ctxt_p[a], ctx_ps)
            nc.vector.tensor_copy(den_p[a], den_ps)
            dent_ps = att.tile([P, 4], F32, tag="den", bufs=1,
                               name=f"dent_ps{a}")
            for qb2 in range(4):
                nc.tensor.matmul(dent_ps[:, qb2:qb2 + 1],
                                 lhsT=den_p[a][0:1, P * qb2:P * (qb2 + 1)],
                                 rhs=ones[0:1, 0:1],
                                 start=(qb2 == 0), stop=(qb2 == 3),
                                 skip_group_check=True)
            nc.vector.reciprocal(rden_p[a], dent_ps)

        def out_block(qb, att, opool, split_store=False, op_bufs=2,
                      store_eng=None, dve_only=False):
            """Output projection for 128-q block qb. 1/den scaling fuses
            into the mandatory psum->SBUF copies, split evenly ACT/DVE.
            split_store stores each half as soon as it's scaled, halves
            on alternating queues (kernel-tail block)."""
            a, qb2 = qb // 4, qb % 4
            if a == 0:
                rd = rden_p[a][:, qb2:qb2 + 1]
            else:
                rd = rdenB[qb2 // 2][:, qb2 % 2:qb2 % 2 + 1]
            ctxb = ctxt_p[a][:, P * qb2:P * (qb2 + 1)]
            ot = opool.tile([P, D_MODEL], F16, tag="ot")
            store_eng = store_eng or nc.sync
            for t in range(2):
                ps = att.tile([P, 512], F32, tag="op", bufs=op_bufs,
                              name=f"op{qb}_{t}")
                nc.tensor.matmul(ps, lhsT=ctxb,
                                 rhs=wo_sb[:, 512 * t:512 * (t + 1)],
                                 start=True, stop=True,
                                 skip_group_check=True)
                if t == 1 and not dve_only:
                    nc.scalar.mul(ot[:, 512 * t:512 * (t + 1)], ps, rd)
                else:
                    nc.vector.tensor_scalar_mul(
                        ot[:, 512 * t:512 * (t + 1)], ps, rd)
                if split_store:
                    eng = nc.sync if t == 0 else nc.scalar
                    eng.dma_start(
                        out=out_d[P * qb:P * (qb + 1),
                                  512 * t:512 * (t + 1)],
                        in_=ot[:, 512 * t:512 * (t + 1)])
            if not split_store:
                store_eng.dma_start(out=out_d[P * qb:P * (qb + 1), :], in_=ot)

        # ---------------- schedule ----------------
        # The HAM clock gate drops the core to half speed after ~1us of
        # low PE duty and takes ~3.4us of sustained activity to
        # restore: every phase seam (psum-copy latency) must be padded
        # with dep-free dummy matmuls or the whole next phase runs 2x
        # slow. "bridge" keeps one psum bank for that purpose.
        with tc.tile_pool(name="bridge", bufs=1, space="PSUM") as brp:
            br_ps = brp.tile([P, 512], F32, name="bridge_ps")

            def bridge(n):
                for _ in range(n):
                    nc.tensor.matmul(br_ps, lhsT=warm_sb[:, 0:P],
                                     rhs=warm_sb, start=True, stop=True,
                                     skip_group_check=True)

            # PE warm-up: covers the first-chunk DMA latency and lifts
            # the clock before the projections begin.
            bridge(WARM_N)
            # NOTE: do NOT emit bridge dummies BETWEEN proj chunks --
            # interleaving foreign matmuls inside the accumulation
            # phase measured +15us (80us total), mechanism unclear.
            with tc.tile_pool(name="pjA", bufs=1, space="PSUM") as pjA:
                proj_half(0, pjA)
            bridge(5)
            with tc.tile_pool(name="trA", bufs=1, space="PSUM") as trA:
                transposes(0, trA)
            bridge(3)
            # attention pair 0 immediately: only needs half-0 K/V/Q.
            # x half 1 streams in underneath it.
            with tc.tile_pool(name="attA", bufs=1, space="PSUM") as attA:
                attention_pair(0, attA)
            bridge(3)
            # pair-A out blocks BEFORE pjB: their scale copies must
            # clear DVE/ACT (and free the op banks) early -- attB
            # needs all 8 psum banks and emitting them after pjB made
            # attB wait on the very last scale copy (measured +8us,
            # HAM drop). Their matmuls also fill pjB's arrival gaps.
            with (
                tc.tile_pool(name="opA", bufs=1, space="PSUM") as opA,
                tc.tile_pool(name="pjB", bufs=1, space="PSUM") as pjB,
            ):
                for qb in range(4):
                    out_block(qb, opA, outA_pool)
                proj_half(1, pjB)
            bridge(3)
            with tc.tile_pool(name="trB", bufs=1, space="PSUM") as trB:
                transposes(1, trB)
            bridge(4)
        # bridge bank freed: attB needs all 8 (st 4 + den 1 + ctx 1 +
        # op 2, the op rotation shared by dent tiles and the early lo
        # out-blocks). Short (~0.8us) seams from here on don't trip
        # the HAM throttle.
        with tc.tile_pool(name="attB", bufs=1, space="PSUM") as attB:

            def earlyB(den_ps, ctx_ps):
                """Slot-2 (lo) finish, emitted while group 3 still
                runs: den/ctx lo columns are final after denav(2)."""
                nc.vector.tensor_copy(den_p[1][:, 0:256],
                                      den_ps[:, 0:256])
                nc.vector.tensor_copy(ctxt_p[1][:, 0:256],
                                      ctx_ps[:, 0:256])
                dent_lo = attB.tile([P, 512], F32, tag="op", bufs=2,
                                    name="dent_lo")
                for j in range(2):
                    nc.tensor.matmul(dent_lo[:, j:j + 1],
                                     lhsT=den_p[1][0:1, P * j:P * (j + 1)],
                                     rhs=ones[0:1, 0:1],
                                     start=(j == 0), stop=(j == 1),
                                     skip_group_check=True)
                nc.vector.reciprocal(rdenB[0], dent_lo[:, 0:2])
                # t1 scales on ACT: they queue right behind its last
                # exp; dve_only serialized 4 copies on DVE (+1.5us)
                for qb in (4, 5):
                    out_block(qb, attB, outB_pool, store_eng=nc.sync)

            attention_pair(1, attB, early_cb=earlyB)
        with tc.tile_pool(name="attB2", bufs=1, space="PSUM") as attB2:
            # tail blocks: both split-stored (each half goes out as
            # soon as it's scaled, t0 on sync / t1 on scalar) so the
            # final drain is two ~256KB transfers per queue instead of
            # a serialized ~1MB; op_bufs=4 removes the qb7-t1 op-bank
            # WAR gap (attB is closed, banks are free)
            out_block(6, attB2, outB_pool, split_store=True, op_bufs=4)
            out_block(7, attB2, outB_pool, split_store=True, op_bufs=4)


_NC_CACHE = None


def build_nc() -> bass.Bass:
    global _NC_CACHE
    if _NC_CACHE is not None:
        return _NC_CACHE
    nc = bacc.Bacc("TRN2", target_bir_lowering=False, debug=False)
    xt_d = nc.dram_tensor("xt", [D_MODEL, SEQ], F16, kind="ExternalInput").ap()
    wq_d = nc.dram_tensor("wq", [P, DC * D_HEAD], F16, kind="ExternalInput").ap()
    wk_d = nc.dram_tensor("wk", [P, DC * D_HEAD], F16, kind="ExternalInput").ap()
    wv_d = nc.dram_tensor("wv", [P, DC * D_HEAD], F16, kind="ExternalInput").ap()
    wo_d = nc.dram_tensor("wo", [D_HEAD, D_MODEL], F16, kind="ExternalInput").ap()
    maskt_d = nc.dram_tensor("maskt", [P, 4 * QW], F16, kind="ExternalInput").ap()
    out_d = nc.dram_tensor("out", [NQ, D_MODEL], F16, kind="ExternalOutput").ap()
    with tile.TileContext(nc) as tc:
        _attention_kernel(tc, xt_d, wq_d, wk_d, wv_d, wo_d, maskt_d, out_d)
    nc.compile()
    _NC_CACHE = nc
    return nc


def _chunk_major(w):
    """[1024, 128] -> [128, 8*128]: row p holds chunks c of w[128c+p, :]."""
    return np.ascontiguousarray(
        w.reshape(DC, P, D_HEAD).transpose(1, 0, 2).reshape(P, DC * D_HEAD))


def kernel(x, wq, wk, wv, wo, _trace=False, _trace_kwargs=None):
    x = np.asarray(x, dtype=np.float32)
    wq_h = _chunk_major(np.asarray(wq, dtype=np.float32).astype(np.float16))
    wk_h = _chunk_major(np.asarray(wk, dtype=np.float32).astype(np.float16))
    wv_h = _chunk_major(np.asarray(wv, dtype=np.float32).astype(np.float16))
    wo_h = np.ascontiguousarray(np.asarray(wo, dtype=np.float32).astype(np.float16))

    nc = build_nc()

    masks = {}
    for p in (0, 1):
        m = (1.0 - make_mask01(p).astype(np.float32)) * MASK_NEG  # additive
        m = m.astype(np.float16)  # [512 k, 256 q]
        masks[p] = np.ascontiguousarray(
            m.reshape(4, P, QW).transpose(1, 0, 2).reshape(P, 4 * QW))
    in_maps = []
    for core in range(NCORES):
        b, parity = core // 2, core % 2
        order = block_order(parity)
        perm = np.concatenate([np.arange(P) + P * o for o in order])
        xt = np.ascontiguousarray(x[b][perm, :].T.astype(np.float16))
        in_maps.append({
            "xt": xt, "wq": wq_h, "wk": wk_h, "wv": wv_h, "wo": wo_h,
            "maskt": masks[parity],
        })

    res = run_bass_kernel_spmd(
        nc, in_maps, core_ids=list(range(NCORES)),
        trace=_trace, **(_trace_kwargs or {}))

    out = np.empty_like(x)
    for core in range(NCORES):
        b, parity = core // 2, core % 2
        order = block_order(parity)
        core_out = res.results[core]["out"].astype(np.float32)
        for j in range(NSLOT):
            for i in range(2):
                qb = order[4 * j + i]
                out[b, P * qb:P * (qb + 1), :] = \
                    core_out[QW * j + P * i:QW * j + P * (i + 1), :]
    if _trace:
        return out, res
    return out
